# revision 1
# baseline (speedup 1.0000x reference)
"""Trainium2 Bass kernel for nn_Net_21818433863930 (interaction-network GNN).

Contract: kernel(**inputs) takes FULL unsharded fp32 inputs (z: (8192, 8, 16) plus
all MLP weights), shards batch across 8 NeuronCores (pure data parallel), runs a
Bass/Tile kernel per core, and returns the FULL (8192, 8, 32) fp32 output.

Architecture notes (per core, n_loc=1024 samples, C=8192 object-cols, feature-major):
  - everything is computed feature-major: SBUF tiles are (features, columns)
  - pair grid (sample, i, j) processed as 16 chunks x 512 object-cols x 9 j-slabs
    (slabs 0..7 = partner j, slab 8 = diagonal pairs with NEGATED aggregation
    weights, which implements the (1 - eye) mask exactly by cancellation)
  - rel-MLP layer 3 commutes with the attention-weighted sum over j, so the
    j-aggregation IS a PSUM-accumulated matmul with lhsT=[rw2; rb2]
  - dist = sqrt(r_i + r_j - 2*G + eps) assembled by PSUM accumulation on the PE
"""
import numpy as np

N, O, CL = 8192, 8, 32
D_IN = 16
EPS = 1e-12
N_CORES = 8
TC = 512  # columns per chunk

_F32 = None  # set lazily (mybir.dt.float32)


# ---------------------------------------------------------------- host packing
def pack_weights(inp: dict) -> dict:
    """Pack reference weights into lhsT/bias arrays the kernel consumes.
    All matmul weights are (K, M) = (in_features, out_features) fp32."""
    f32 = np.float32
    w = {}
    w["wenc"] = np.ascontiguousarray(inp["Wenc"], f32)            # (16, 32)
    w["benc"] = np.ascontiguousarray(inp["benc"].reshape(32, 1), f32)

    for c in range(3):
        rw0, aw0 = inp["rw0"][c], inp["aw0"][c]                   # (65, 64)
        w[f"w1pa_{c}"] = np.concatenate([rw0[0:32], aw0[0:32]], axis=1).astype(f32)
        w[f"w1pb_{c}"] = np.concatenate([rw0[32:64], aw0[32:64]], axis=1).astype(f32)
        w[f"w1pc_{c}"] = np.concatenate([rw0[64:65], aw0[64:65]], axis=1).astype(f32)
        w[f"b1_{c}"] = np.concatenate([inp["rb0"][c], inp["ab0"][c]]).reshape(128, 1).astype(f32)

        # a1 rows: 0:64 rel1, 64:128 att1.
        # a2 rows: 0:32 m(rel2), 32 ones, 33:64 zero, 64:96 t(att2)
        # (everything an elementwise op touches is base-partition aligned)
        w2p = np.zeros((128, 96), f32)
        w2p[0:64, 0:32] = inp["rw1"][c]                            # rel branch -> m
        w2p[64:128, 64:96] = inp["aw1"][c]                         # att branch -> t
        w[f"w2p_{c}"] = w2p
        b2 = np.zeros((96, 1), f32)
        b2[0:32, 0] = inp["rb1"][c]; b2[32, 0] = 1.0; b2[64:96, 0] = inp["ab1"][c]
        w[f"b2_{c}"] = b2

        w3a = np.zeros((96, 33), f32)
        w3a[64:96, :] = np.tile(inp["aw2"][c].reshape(32, 1), (1, 33))
        w[f"w3a_{c}"] = w3a
        w[f"ab2r_{c}"] = np.full((33, 1), float(inp["ab2"][c][0]), f32)

        w3r = np.zeros((33, 32), f32)
        w3r[0:32] = inp["rw2"][c]; w3r[32] = inp["rb2"][c]
        w[f"w3r_{c}"] = w3r

    def bd(ms):  # block-diag of three (32, 32)
        out = np.zeros((96, 96), f32)
        for c in range(3):
            out[32*c:32*c+32, 32*c:32*c+32] = ms[c]
        return out

    w["wself0s"] = np.concatenate([inp["sw0"][c] for c in range(3)], axis=1).astype(f32)  # (32, 96)
    w["sb0s"] = np.concatenate([inp["sb0"][c] for c in range(3)]).reshape(96, 1).astype(f32)
    w["wself1bd"] = bd([inp["sw1"][c] for c in range(3)])
    w["dynb"] = np.concatenate([inp["sb1"][c] for c in range(3)]).reshape(96, 1).astype(f32)

    for k, (wn, bn) in enumerate([("fw0", "fb0"), ("fw1", "fb1"), ("fw2", "fb2")]):
        w[f"waff{k}bd"] = bd([inp[wn][c] for c in range(3)])
        w[f"fb{k}s"] = np.concatenate([inp[bn][c] for c in range(3)]).reshape(96, 1).astype(f32)

    w["wow0abd"] = bd([inp["ow0"][c][0:32] for c in range(3)])
    w["wow0ss"] = np.concatenate([inp["ow0"][c][32:64] for c in range(3)], axis=1).astype(f32)  # (32, 96)
    w["ob0s"] = np.concatenate([inp["ob0"][c] for c in range(3)]).reshape(96, 1).astype(f32)
    w["wow1bd"] = bd([inp["ow1"][c] for c in range(3)])

    ob1cat = np.concatenate([inp["ob1"][c] for c in range(3)])     # (96,)
    w["wagg1"] = np.ascontiguousarray(inp["Wagg1"], f32)           # (96, 32)
    w["bagg1p"] = (inp["Wagg1"].T @ ob1cat + inp["bagg1"]).reshape(32, 1).astype(f32)
    w["wagg2"] = np.ascontiguousarray(inp["Wagg2"], f32)           # (32, 32)
    w["bagg2"] = np.ascontiguousarray(inp["bagg2"].reshape(32, 1), f32)

    w["ones32"] = np.ones((32, 1), f32)
    w["negtwo32"] = np.full((32, 1), -2.0, f32)
    for c in range(3):
        w3rp = np.zeros((33, 96), f32)
        w3rp[:, 32*c:32*c+32] = w[f"w3r_{c}"]
        w[f"w3rp_{c}"] = w3rp; w[f"w3rn_{c}"] = -w3rp
        del w[f"w3r_{c}"]
    # d2 = -2*sum(prod) + r_j + r_i reduction weights
    dw33 = np.full((33, 1), -2.0, f32); dw33[32] = 1.0
    w["dw33"] = dw33
    w["one1"] = np.ones((1, 1), f32)
    onep32 = np.zeros((33, 1), f32); onep32[32] = 1.0
    w["onep32"] = onep32  # lhsT slice [32:33] -> 1.0 at base partition 32
    w["epsb"] = np.full((1, 1), EPS, f32)
    return w


WEIGHT_SPECS = None  # filled by build_nc


# ---------------------------------------------------------------- device build
def build_nc(n_loc: int, repeat: int = 1, am_engine: str = 'gpsimd', a1_engine: str = 'act', bufs_sbuf: int = 3):
    """Build + compile the per-core Bass program. n_loc = samples per core."""
    import concourse.bass as bass
    import concourse.bacc as bacc
    import concourse.tile as tile
    import concourse.mybir as mybir
    from contextlib import ExitStack

    f32 = mybir.dt.float32
    f32r = mybir.dt.float16  # compute dtype: full PE rate, 2x DVE modes
    AF = mybir.ActivationFunctionType
    OP = mybir.AluOpType

    def mm(out, lhsT, rhs, **kw):
        nc.tensor.matmul(out, lhsT, rhs, **kw)

    C = n_loc * O
    n_chunks = C // TC
    assert C % TC == 0

    nc = bacc.Bacc("TRN2", target_bir_lowering=False, debug=False,
                   enable_asserts=False)

    zT = nc.dram_tensor("zT", (D_IN, C), f32r, kind="ExternalInput").ap()
    outT = nc.dram_tensor("outT", (CL, C), f32, kind="ExternalOutput").ap()

    wspecs = {
        "wenc": (16, 32), "benc": (32, 1),
        "wself0s": (32, 96), "sb0s": (96, 1), "wself1bd": (96, 96), "dynb": (96, 1),
        "waff0bd": (96, 96), "fb0s": (96, 1), "waff1bd": (96, 96), "fb1s": (96, 1),
        "waff2bd": (96, 96), "fb2s": (96, 1),
        "wow0abd": (96, 96), "wow0ss": (32, 96), "ob0s": (96, 1), "wow1bd": (96, 96),
        "wagg1": (96, 32), "bagg1p": (32, 1), "wagg2": (32, 32), "bagg2": (32, 1),
        "ones32": (32, 1), "negtwo32": (32, 1), "one1": (1, 1), "onep32": (33, 1), "epsb": (1, 1),
    }
    for c in range(3):
        wspecs[f"w1pa_{c}"] = (32, 128); wspecs[f"w1pb_{c}"] = (32, 128); wspecs[f"w1pc_{c}"] = (1, 128); wspecs[f"b1_{c}"] = (128, 1)
        wspecs[f"w2p_{c}"] = (128, 96); wspecs[f"b2_{c}"] = (96, 1)
        wspecs[f"w3a_{c}"] = (96, 33); wspecs[f"ab2r_{c}"] = (33, 1)
    for c in range(3):
        wspecs[f"w3rp_{c}"] = (33, 96); wspecs[f"w3rn_{c}"] = (33, 96)
    wspecs["epsrow"] = (1, C)
    wspecs["dw33"] = (33, 1)
    global WEIGHT_SPECS
    WEIGHT_SPECS = wspecs

    MMW = set(['wenc', 'wself0s', 'wself1bd', 'waff0bd', 'waff1bd', 'waff2bd', 'wow0abd', 'wow0ss', 'wow1bd', 'wagg1', 'wagg2', 'ones32', 'negtwo32', 'one1', 'dw33', 'epsrow', 'w1pa_0', 'w1pa_1', 'w1pa_2', 'w1pb_0', 'w1pb_1', 'w1pb_2', 'w1pc_0', 'w1pc_1', 'w1pc_2', 'w2p_0', 'w2p_1', 'w2p_2', 'w3a_0', 'w3a_1', 'w3a_2', 'w3rp_0', 'w3rp_1', 'w3rp_2', 'w3rn_0', 'w3rn_1', 'w3rn_2'])
    wdram = {k: nc.dram_tensor(k, shp, f32r if k in MMW else f32,
                               kind="ExternalInput").ap()
             for k, shp in wspecs.items()}

    with tile.TileContext(nc) as tc:
        with ExitStack() as ctx:
            # --- persistent weight tiles
            wpool = ctx.enter_context(tc.tile_pool(name="w", bufs=1))
            W = {}
            for k, shp in wspecs.items():
                W[k] = wpool.tile(list(shp), f32r if k in MMW else f32,
                                  tag=k, name=f"w_{k}")
                nc.sync.dma_start(W[k][:], wdram[k][:])

            # persistent tensors
            pers = ctx.enter_context(tc.tile_pool(name="pers", bufs=1))
            stf = pers.tile([32, C], f32r, tag="stf", name="stf")
            aux = pers.tile([O, C], f32r, tag="aux", name="aux")  # rows 0:8 = dist_j

            # --- pools
            p_zt = ctx.enter_context(tc.tile_pool(name="zt", bufs=2))
            p_sq = ctx.enter_context(tc.tile_pool(name="sq", bufs=2))
            p_hself = ctx.enter_context(tc.tile_pool(name="hself", bufs=2))
            p_x = ctx.enter_context(tc.tile_pool(name="x", bufs=3))
            p_prod = ctx.enter_context(tc.tile_pool(name="prod", bufs=2))
            p_dt = ctx.enter_context(tc.tile_pool(name="dt", bufs=2))
            p_a1 = ctx.enter_context(tc.tile_pool(name="a1", bufs=3))
            p_a2 = ctx.enter_context(tc.tile_pool(name="a2", bufs=3))
            p_al = ctx.enter_context(tc.tile_pool(name="al", bufs=3))
            p_am = ctx.enter_context(tc.tile_pool(name="am", bufs=3))
            p_ep = ctx.enter_context(tc.tile_pool(name="ep", bufs=4))
            p_out = ctx.enter_context(tc.tile_pool(name="outs", bufs=2))

            ps_A = ctx.enter_context(tc.tile_pool(name="psA", bufs=2, space="PSUM"))
            ps_B = ctx.enter_context(tc.tile_pool(name="psB", bufs=2, space="PSUM"))
            ps_S = ctx.enter_context(tc.tile_pool(name="psS", bufs=2, space="PSUM"))
            ps_D = ctx.enter_context(tc.tile_pool(name="psD", bufs=2, space="PSUM"))

            for _rep in range(repeat):
              # ============ PASS 0: encoder + r + dist (sqrt table resident)
              for cc in range(n_chunks):
                cs = cc * TC
                s0 = cs // O          # first sample of chunk
                ns = TC // O          # samples per chunk
                zt = p_zt.tile([D_IN, TC], f32r, tag="zt", name="zt")
                nc.sync.dma_start(zt[:], zT[:, cs:cs+TC])
                ps_enc = ps_S.tile([33, TC], f32, tag="ps_s", name="ps_enc")
                mm(ps_enc[0:32, :], W["wenc"][:], zt[:], start=True, stop=True)
                nc.vector.tensor_scalar(stf[0:32, cs:cs+TC], ps_enc[0:32, :],
                                        W["benc"][:], None, op0=OP.add)
                st_soi = stf[0:32, :].rearrange("p (s i) -> p s i", i=O)
                for j in range(O):
                    diff = p_sq.tile([32, TC], f32r, tag="diff", name="diff")
                    nc.vector.tensor_tensor(
                        diff[:].rearrange("p (s i) -> p s i", i=O),
                        st_soi[:, s0:s0+ns, :],
                        st_soi[:, s0:s0+ns, j:j+1].broadcast_to((32, ns, O)),
                        op=OP.subtract)
                    dsq = p_prod.tile([32, TC], f32r, tag="dsq", name="dsq")
                    nc.vector.tensor_tensor(dsq[:], diff[:], diff[:], op=OP.mult)
                    ps_d2 = ps_S.tile([33, TC], f32, tag="ps_s", name="ps_d2")
                    mm(ps_d2[0:1, :], W["ones32"][:], dsq[:], start=True, stop=True)
                    dtmp = p_dt.tile([1, TC], f32r, tag="dtmp", name="dtmp")
                    nc.scalar.activation(dtmp[:], ps_d2[0:1, :], AF.Sqrt, bias=W["epsb"][:])
                    nc.gpsimd.dma_start(aux[j:1+j, cs:cs+TC], dtmp[:])

              # ============ MAIN PASS (sigmoid table resident)
              for cc in range(n_chunks):
                cs = cc * TC
                s0 = cs // O
                ns = TC // O
                st_soi = stf[0:32, :].rearrange("p (s i) -> p s i", i=O)
                hself = p_hself.tile([96, TC], f32r, tag="hself", name="hself")
                ps_h = ps_D.tile([96, TC], f32, tag="ps_d", name="ps_h")
                mm(ps_h[:], W["wself0s"][:], stf[0:32, cs:cs+TC], start=True, stop=True)
                nc.vector.tensor_scalar(hself[:], ps_h[:], W["sb0s"][:], 0.0,
                                        op0=OP.add, op1=OP.max)

                dyn_acc = ps_D.tile([96, TC], f32, tag="ps_d", name="dyn_acc")

                for j in range(O + 1):
                    if j < O:
                        xjt = p_x.tile([32, TC], f32r, tag="x", name="xjt")
                        nc.gpsimd.tensor_copy(
                            xjt[:].rearrange("p (s i) -> p s i", i=O),
                            st_soi[:, s0:s0+ns, j:j+1].broadcast_to((32, ns, O)))
                        xb = xjt[:, :]
                        xd = p_dt.tile([1, TC], f32r, tag="xd", name="xd")
                        nc.scalar.dma_start(xd[:], aux[j:1+j, cs:cs+TC])
                        xdist = xd[:, :]
                    else:  # diagonal slab: partner = self, dist = sqrt(eps)
                        xb = stf[0:32, cs:cs+TC]
                        xdist = W["epsrow"][0:1, cs:cs+TC]

                    for c in range(3):
                        psA = ps_A.tile([128, TC], f32, tag="psA", name="psA")
                        mm(psA[:], W[f"w1pa_{c}"][:], stf[0:32, cs:cs+TC],
                           start=True, stop=False)
                        mm(psA[:], W[f"w1pb_{c}"][:], xb, start=False, stop=False)
                        mm(psA[:], W[f"w1pc_{c}"][:], xdist, start=False, stop=True)
                        a1 = p_a1.tile([128, TC], f32r, tag="a1", name="a1")
                        nc.scalar.activation(a1[:], psA[:], AF.Relu, bias=W[f"b1_{c}"][:])
                        psB = ps_B.tile([96, TC], f32, tag="psB", name="psB")
                        mm(psB[:], W[f"w2p_{c}"][:], a1[:], start=True, stop=True)
                        a2 = p_a2.tile([96, TC], f32r, tag="a2", name="a2")
                        nc.vector.tensor_scalar(a2[:], psB[:], W[f"b2_{c}"][:], 0.0,
                                                op0=OP.add, op1=OP.max)
                        psC = ps_S.tile([33, TC], f32, tag="ps_s", name="psC")
                        mm(psC[:], W[f"w3a_{c}"][64:96, :], a2[64:96, :], start=True, stop=True)
                        alr = p_al.tile([33, TC], f32r, tag="al", name="alr")
                        nc.scalar.activation(alr[:], psC[:], AF.Sigmoid, bias=W[f"ab2r_{c}"][:])
                        am = p_am.tile([33, TC], f32r, tag="am", name="am")
                        nc.vector.tensor_tensor(am[:], a2[0:33, :], alr[:], op=OP.mult)
                        wkey = f"w3rp_{c}" if j < O else f"w3rn_{c}"
                        mm(dyn_acc[:], W[wkey][:], am[:],
                           start=(j == 0 and c == 0), stop=False,
                           skip_group_check=True)

                # self-dynamics into the same accumulator, then evacuate
                mm(dyn_acc[:], W["wself1bd"][:], hself[:],
                   start=False, stop=True, skip_group_check=True)
                dyn = p_ep.tile([96, TC], f32r, tag="ep", name="dyn")
                nc.vector.tensor_scalar(dyn[:], dyn_acc[:], W["dynb"][:], None, op0=OP.add)

                # ---- affector + out + agg chains
                cur = dyn
                for k in range(3):
                    psE = ps_D.tile([96, TC], f32, tag="ps_d", name="psE")
                    mm(psE[:], W[f"waff{k}bd"][:], cur[:], start=True, stop=True)
                    nxt = p_ep.tile([96, TC], f32r, tag="ep", name="nxt")
                    if k < 2:
                        nc.vector.tensor_scalar(nxt[:], psE[:], W[f"fb{k}s"][:], 0.0,
                                                op0=OP.add, op1=OP.max)
                    else:
                        nc.vector.tensor_scalar(nxt[:], psE[:], W[f"fb{k}s"][:], None, op0=OP.add)
                    cur = nxt
                psO = ps_D.tile([96, TC], f32, tag="ps_d", name="psO")
                mm(psO[:], W["wow0abd"][:], cur[:], start=True, stop=False)
                mm(psO[:], W["wow0ss"][:], stf[0:32, cs:cs+TC], start=False, stop=True)
                o0 = p_ep.tile([96, TC], f32r, tag="ep", name="o0")
                nc.vector.tensor_scalar(o0[:], psO[:], W["ob0s"][:], 0.0,
                                        op0=OP.add, op1=OP.max)
                psO1 = ps_D.tile([96, TC], f32, tag="ps_d", name="psO1")
                mm(psO1[:], W["wow1bd"][:], o0[:], start=True, stop=True)
                ccat = p_ep.tile([96, TC], f32r, tag="ep", name="ccat")
                nc.vector.tensor_scalar(ccat[:], psO1[:], 0.0, None, op0=OP.add)
                psG = ps_S.tile([33, TC], f32, tag="ps_s", name="psG")
                mm(psG[0:32, :], W["wagg1"][:], ccat[:], start=True, stop=True)
                h = p_ep.tile([32, TC], f32r, tag="ep", name="h")
                nc.vector.tensor_scalar(h[:], psG[0:32, :], W["bagg1p"][:], 0.0,
                                        op0=OP.add, op1=OP.max)
                psG2 = ps_S.tile([33, TC], f32, tag="ps_s", name="psG2")
                mm(psG2[0:32, :], W["wagg2"][:], h[:], start=True, stop=True)
                ot = p_out.tile([32, TC], f32, tag="ot", name="ot")
                nc.vector.tensor_scalar(ot[:], psG2[0:32, :], W["bagg2"][:], None, op0=OP.add)
                nc.sync.dma_start(outT[:, cs:cs+TC], ot[:])

    nc.compile()
    return nc


# ---------------------------------------------------------------- host runner
_CACHE = {}


def _make_runner(nc, n_cores=N_CORES):
    import jax
    import numpy as _np
    import concourse.mybir as mybir
    from concourse import bass2jax
    from jax.sharding import Mesh, PartitionSpec
    from jax.experimental.shard_map import shard_map

    bass2jax.install_neuronx_cc_hook()
    partition_name = nc.partition_id_tensor.name if nc.partition_id_tensor else None
    in_names, out_names, out_avals, zero_shapes = [], [], [], []
    for alloc in nc.m.functions[0].allocations:
        if not isinstance(alloc, mybir.MemoryLocationSet):
            continue
        name = alloc.memorylocations[0].name
        if alloc.kind == "ExternalInput":
            if name != partition_name:
                in_names.append(name)
        elif alloc.kind == "ExternalOutput":
            out_names.append(name)
            shape = tuple(alloc.tensor_shape)
            dtype = mybir.dt.np(alloc.dtype)
            out_avals.append(jax.core.ShapedArray(shape, dtype))
            zero_shapes.append((shape, dtype))
    n_params = len(in_names)
    n_outs = len(out_avals)
    all_in_names = in_names + out_names + ([partition_name] if partition_name else [])
    donate = tuple(range(n_params, n_params + n_outs))

    def _body(*args):
        operands = list(args)
        if partition_name is not None:
            operands.append(bass2jax.partition_id_tensor())
        outs = bass2jax._bass_exec_p.bind(
            *operands, out_avals=tuple(out_avals), in_names=tuple(all_in_names),
            out_names=tuple(out_names), lowering_input_output_aliases=(),
            sim_require_finite=False, sim_require_nnan=False, nc=nc)
        return tuple(outs)

    devices = jax.devices()[:n_cores]
    mesh = Mesh(_np.asarray(devices), ("core",))
    sharded = jax.jit(
        shard_map(_body, mesh=mesh,
                  in_specs=(PartitionSpec("core"),) * (n_params + n_outs),
                  out_specs=(PartitionSpec("core"),) * n_outs,
                  check_rep=False),
        donate_argnums=donate, keep_unused=True)

    def run(in_maps):
        per_core = [[_np.asarray(m[name]) for name in in_names] for m in in_maps]
        concat_in = [_np.concatenate([per_core[c][i] for c in range(n_cores)], axis=0)
                     for i in range(n_params)]
        concat_zeros = [_np.zeros((n_cores * s[0], *s[1:]), d) for s, d in zero_shapes]
        out_arrs = sharded(*concat_in, *concat_zeros)
        jax.block_until_ready(out_arrs)
        return [
            {name: _np.asarray(out_arrs[i]).reshape(n_cores, *out_avals[i].shape)[c]
             for i, name in enumerate(out_names)}
            for c in range(n_cores)
        ]
    return run


_MMW = ['wenc', 'wself0s', 'wself1bd', 'waff0bd', 'waff1bd', 'waff2bd',
        'wow0abd', 'wow0ss', 'wow1bd', 'wagg1', 'wagg2', 'ones32', 'negtwo32',
        'one1', 'dw33', 'epsrow'] +        [f"w1pa_{c}" for c in range(3)] + [f"w1pb_{c}" for c in range(3)] +        [f"w1pc_{c}" for c in range(3)] + [f"w2p_{c}" for c in range(3)] +        [f"w3a_{c}" for c in range(3)] + [f"w3rp_{c}" for c in range(3)] +        [f"w3rn_{c}" for c in range(3)]


def make_in_maps(inputs: dict, n_loc: int, n_cores: int = N_CORES):
    w = pack_weights(inputs)
    for k in _MMW:
        if k in w:
            w[k] = w[k].astype(np.float16)
    z = np.asarray(inputs["z"], np.float32)
    in_maps = []
    for c in range(n_cores):
        zc = z[c*n_loc:(c+1)*n_loc].reshape(n_loc * O, D_IN)
        m = dict(w)
        m["zT"] = np.ascontiguousarray(zc.T).astype(np.float16)
        m["epsrow"] = np.full((1, n_loc * O), 1e-6, np.float16)
        in_maps.append(m)
    return in_maps


def kernel(**inputs) -> np.ndarray:
    n = inputs["z"].shape[0]
    n_loc = n // N_CORES
    key = ("k", n_loc)
    if key not in _CACHE:
        nc = build_nc(n_loc)
        _CACHE[key] = (nc, _make_runner(nc))
    nc, runner = _CACHE[key]
    res = runner(make_in_maps(inputs, n_loc))
    out = np.concatenate(
        [res[c]["outT"].T.reshape(n_loc, O, CL) for c in range(N_CORES)], axis=0)
    return out



# revision 24
# speedup vs baseline: 7.2510x; 7.2510x over previous
"""Trainium2 Bass kernel for nn_Net_21818433863930 (interaction-network GNN).

Contract: kernel(**inputs) takes FULL unsharded fp32 inputs (z: (8192, 8, 16) plus
all MLP weights), shards batch across 8 NeuronCores (pure data parallel), runs a
Bass/Tile kernel per core, and returns the FULL (8192, 8, 32) fp32 output.

v2 dataflow (per core, n_loc=1024 samples, C=8192 object-cols, feature-major):
  - pass 0 (sqrt act-table): encoder + all pair distances; the 8 per-j squared
    distances accumulate as rows of ONE psum tile so a single Sqrt per chunk
    produces the whole (8, TC) distance block, kept SBUF-resident in `aux`.
  - main pass (sigmoid table), per chunk x 8 j-slabs:
      * rhs65 = [s_i; s_j(bcast); dist_j] packed K=65 -> ONE matmul per core
        for pair-MLP layer 1 (vs 3 accumulating matmuls)
      * the three cores' attention logits go to one 99-row psum tile via
        33-wide tiled aw2 lhsT -> ONE sigmoid per j
      * gated products am3 (3 cores stacked, 99 rows incl. bias-ones rows)
        -> ONE block-diagonal aggregation matmul (K=99) per j
      * diagonal (i==j) mask implemented by an extra tiny matmul on the
        stride-8 diagonal columns with negated weights: exact cancellation
  - PSUM evacuations are spread round-robin over Act/DVE/Pool engines.
"""
import numpy as np

N, O, CL = 8192, 8, 32
D_IN = 16
EPS = 1e-12
N_CORES = 8
TC = 512  # columns per chunk

# engine cycle estimates (ns) for documentation only
# PE 512-col mm ~213, Act evac ~612, DVE psum evac ~658, Pool copy ~806


# ---------------------------------------------------------------- host packing
def pack_weights(inp: dict) -> dict:
    """Pack reference weights into lhsT/bias arrays the kernel consumes.
    All matmul weights are (K, M) = (in_features, out_features) fp32."""
    f32 = np.float32
    w = {}
    w["wenc"] = np.ascontiguousarray(inp["Wenc"], f32)            # (16, 32)
    w["benc"] = np.ascontiguousarray(inp["benc"].reshape(32, 1), f32)

    for c in range(3):
        rw0, aw0 = inp["rw0"][c], inp["aw0"][c]                   # (65, 64)
        # L1: K=65 rows [s_j(32); s_i(32); dist(1)], M=128 [rel1(64) | att1(64)]
        # (s_j first: the per-j broadcast copy then writes at base partition 0)
        w1 = np.concatenate([rw0, aw0], axis=1)                   # (65, 128)
        w[f"w1p_{c}"] = np.concatenate(
            [w1[32:64], w1[0:32], w1[64:65]], axis=0).astype(f32)
        w[f"b1_{c}"] = np.concatenate([inp["rb0"][c], inp["ab0"][c]]).reshape(128, 1).astype(f32)

        # L2 (w2p): K=128 a1, M=96. Per-core row layout keeps every later
        # elementwise op base-partition aligned: rel2pre at rows 32c,
        # att_hidden at rows 32*((c+1)%3).
        rel0, att0 = 32 * c, 32 * ((c + 1) % 3)
        w2p = np.zeros((128, 96), f32)
        w2p[0:64, rel0:rel0+32] = inp["rw1"][c]
        w2p[64:128, att0:att0+32] = inp["aw1"][c]
        w[f"w2p_{c}"] = w2p
        b2 = np.zeros((96, 1), f32)
        b2[rel0:rel0+32, 0] = inp["rb1"][c]; b2[att0:att0+32, 0] = inp["ab1"][c]
        w[f"b2_{c}"] = b2

        # att logit: K=32 (a2 att rows), M=32 tiled -> broadcast over 32 rows
        # rows att0:att0+32 hold the data so lhsT shares rhs's base partition
        w3a = np.zeros((96, 32), f32)
        w3a[att0:att0+32] = np.tile(inp["aw2"][c].reshape(32, 1), (1, 32))
        w[f"w3a_{c}"] = w3a

    ab2r3 = np.zeros((96, 1), f32)
    for c in range(3):
        ab2r3[32*c:32*c+32, 0] = float(inp["ab2"][c][0])
    w["ab2r3"] = ab2r3

    # stacked aggregation: K=96 [am_0(32); am_1(32); am_2(32)], M=96 block-diag
    w3r3 = np.zeros((96, 96), f32)
    for c in range(3):
        w3r3[32*c:32*c+32, 32*c:32*c+32] = inp["rw2"][c]
    w["w3r3"] = w3r3
    w["w3r3n"] = -w3r3
    # rb2 * att_c term: read one replicated att row per core from alr3
    w3rb = np.zeros((96, 96), f32)
    for c in range(3):
        w3rb[32*c, 32*c:32*c+32] = inp["rb2"][c]
    w["w3rb"] = w3rb
    w["w3rbn"] = -w3rb

    def bd(ms):  # block-diag of three (32, 32)
        out = np.zeros((96, 96), f32)
        for c in range(3):
            out[32*c:32*c+32, 32*c:32*c+32] = ms[c]
        return out

    w["wself0s"] = np.concatenate([inp["sw0"][c] for c in range(3)], axis=1).astype(f32)  # (32, 96)
    w["sb0s"] = np.concatenate([inp["sb0"][c] for c in range(3)]).reshape(96, 1).astype(f32)
    w["wself1bd"] = bd([inp["sw1"][c] for c in range(3)])
    w["dynb"] = np.concatenate([inp["sb1"][c] for c in range(3)]).reshape(96, 1).astype(f32)

    for k, (wn, bn) in enumerate([("fw0", "fb0"), ("fw1", "fb1"), ("fw2", "fb2")]):
        w[f"waff{k}bd"] = bd([inp[wn][c] for c in range(3)])
        w[f"fb{k}s"] = np.concatenate([inp[bn][c] for c in range(3)]).reshape(96, 1).astype(f32)

    w["wow0abd"] = bd([inp["ow0"][c][0:32] for c in range(3)])
    w["wow0ss"] = np.concatenate([inp["ow0"][c][32:64] for c in range(3)], axis=1).astype(f32)  # (32, 96)
    w["ob0s"] = np.concatenate([inp["ob0"][c] for c in range(3)]).reshape(96, 1).astype(f32)
    w["wow1bd"] = bd([inp["ow1"][c] for c in range(3)])

    ob1cat = np.concatenate([inp["ob1"][c] for c in range(3)])     # (96,)
    w["wagg1"] = np.ascontiguousarray(inp["Wagg1"], f32)           # (96, 32)
    w["bagg1p"] = (inp["Wagg1"].T @ ob1cat + inp["bagg1"]).reshape(32, 1).astype(f32)
    w["wagg2"] = np.ascontiguousarray(inp["Wagg2"], f32)           # (32, 32)
    w["bagg2"] = np.ascontiguousarray(inp["bagg2"].reshape(32, 1), f32)

    # batched d^2: lhsT (128, 4), col q = ones over partition rows 32q:32q+32
    onesb4 = np.zeros((128, 4), f32)
    for q in range(4):
        onesb4[32*q:32*q+32, q] = 1.0
    w["onesb4"] = onesb4
    w["eps36"] = np.full((36, 1), EPS, f32)
    return w


# fp16 matmul lhsT weights; everything else stays fp32 (biases)
_MMW = (["wenc", "wself0s", "wself1bd", "waff0bd", "waff1bd", "waff2bd",
         "wow0abd", "wow0ss", "wow1bd", "wagg1", "wagg2", "onesb4",
         "w3r3", "w3r3n", "w3rb", "w3rbn"]
        + [f"w1p_{c}" for c in range(3)] + [f"w2p_{c}" for c in range(3)]
        + [f"w3a_{c}" for c in range(3)])

WEIGHT_SPECS = {
    "wenc": (16, 32), "benc": (32, 1),
    "wself0s": (32, 96), "sb0s": (96, 1), "wself1bd": (96, 96), "dynb": (96, 1),
    "waff0bd": (96, 96), "fb0s": (96, 1), "waff1bd": (96, 96), "fb1s": (96, 1),
    "waff2bd": (96, 96), "fb2s": (96, 1),
    "wow0abd": (96, 96), "wow0ss": (32, 96), "ob0s": (96, 1), "wow1bd": (96, 96),
    "wagg1": (96, 32), "bagg1p": (32, 1), "wagg2": (32, 32), "bagg2": (32, 1),
    "onesb4": (128, 4), "eps36": (36, 1),
    "w3r3": (96, 96), "w3r3n": (96, 96), "w3rb": (96, 96), "w3rbn": (96, 96),
    "ab2r3": (96, 1),
}
for _c in range(3):
    WEIGHT_SPECS[f"w1p_{_c}"] = (65, 128)
    WEIGHT_SPECS[f"b1_{_c}"] = (128, 1)
    WEIGHT_SPECS[f"w2p_{_c}"] = (128, 96)
    WEIGHT_SPECS[f"b2_{_c}"] = (96, 1)
    WEIGHT_SPECS[f"w3a_{_c}"] = (96, 32)


# ---------------------------------------------------------------- device build
def build_nc(n_loc: int, repeat: int = 1, bufs_sbuf: int = 3):
    """Build + compile the per-core Bass program. n_loc = samples per core."""
    import concourse.bass as bass
    import concourse.bacc as bacc
    import concourse.tile as tile
    import concourse.mybir as mybir
    from contextlib import ExitStack

    f32 = mybir.dt.float32
    f16 = mybir.dt.float16
    AF = mybir.ActivationFunctionType
    OP = mybir.AluOpType

    C = n_loc * O
    n_chunks = C // TC
    assert C % TC == 0

    nc = bacc.Bacc("TRN2", target_bir_lowering=False, debug=False,
                   enable_asserts=False)

    def mm(out, lhsT, rhs, **kw):
        nc.tensor.matmul(out, lhsT, rhs, **kw)

    zT = nc.dram_tensor("zT", (D_IN, C), f16, kind="ExternalInput").ap()
    outT = nc.dram_tensor("outT", (CL, C), f32, kind="ExternalOutput").ap()

    wdram = {k: nc.dram_tensor(k, shp, f16 if k in _MMW else f32,
                               kind="ExternalInput").ap()
             for k, shp in WEIGHT_SPECS.items()}

    with tile.TileContext(nc) as tc:
        with ExitStack() as ctx:
            # --- persistent weight tiles
            wpool = ctx.enter_context(tc.tile_pool(name="w", bufs=1))
            W = {}
            for k, shp in WEIGHT_SPECS.items():
                W[k] = wpool.tile(list(shp), f16 if k in _MMW else f32,
                                  tag=k, name=f"w_{k}")
                nc.sync.dma_start(W[k][:], wdram[k][:])

            # persistent tensors
            pers = ctx.enter_context(tc.tile_pool(name="pers", bufs=1))
            # s replicated in 4 partition blocks (rows 32q) so the stacked
            # pass-0 diff/dsq blocks stay base-partition aligned
            stf4 = pers.tile([128, C], f16, tag="stf4", name="stf4")
            # dist rows: j<4 at row j, j>=4 at row 28+j (32-aligned mm targets)
            aux = pers.tile([36, C], f16, tag="aux", name="aux")
            rhs65 = [pers.tile([65, TC], f16, tag=f"rhs65_{b}", name=f"rhs65_{b}")
                     for b in range(2)]

            # --- SBUF work pools
            p_zt = ctx.enter_context(tc.tile_pool(name="zt", bufs=2))
            p_df = ctx.enter_context(tc.tile_pool(name="df", bufs=2))
            p_sq = ctx.enter_context(tc.tile_pool(name="sq", bufs=2))
            p_a1 = ctx.enter_context(tc.tile_pool(name="a1", bufs=3))
            p_a2 = ctx.enter_context(tc.tile_pool(name="a2", bufs=3))
            p_al = ctx.enter_context(tc.tile_pool(name="al", bufs=2))
            p_am = ctx.enter_context(tc.tile_pool(name="am", bufs=2))
            p_hs = ctx.enter_context(tc.tile_pool(name="hs", bufs=2))
            p_ep = ctx.enter_context(tc.tile_pool(name="ep", bufs=3))
            p_out = ctx.enter_context(tc.tile_pool(name="outs", bufs=2))

            # --- PSUM pools (8 banks total)
            ps_A = ctx.enter_context(tc.tile_pool(name="psA", bufs=3, space="PSUM"))
            ps_B = ctx.enter_context(tc.tile_pool(name="psB", bufs=3, space="PSUM"))
            ps_C = ctx.enter_context(tc.tile_pool(name="psC", bufs=1, space="PSUM"))
            ps_D = ctx.enter_context(tc.tile_pool(name="psD", bufs=1, space="PSUM"))

            # evac helpers: engine e in {0: Act, 1: DVE}; Pool cannot read PSUM
            def evac_relu(e, dst, src, bias):
                if e == 0:
                    nc.scalar.activation(dst, src, AF.Relu, bias=bias)
                else:
                    nc.vector.tensor_scalar(dst, src, bias, 0.0, op0=OP.add, op1=OP.max)

            def evac_add(e, dst, src, bias):
                if e == 0:
                    nc.scalar.activation(dst, src, AF.Identity, bias=bias)
                else:
                    nc.vector.tensor_scalar(dst, src, bias, None, op0=OP.add)

            for _rep in range(repeat):
              # ============ PASS 0: encoder + distances (sqrt table resident)
              for cc in range(n_chunks):
                cs = cc * TC
                s0 = cs // O          # first sample of chunk
                ns = TC // O          # samples per chunk
                zt = p_zt.tile([D_IN, TC], f16, tag="zt", name="zt")
                nc.sync.dma_start(zt[:], zT[:, cs:cs+TC])
                ps_enc = ps_A.tile([32, TC], f32, tag="psA", name="ps_enc")
                mm(ps_enc[:], W["wenc"][:], zt[:], start=True, stop=True)
                nc.scalar.activation(stf4[0:32, cs:cs+TC], ps_enc[:],
                                     AF.Identity, bias=W["benc"][:])
                for q in range(1, 4):
                    nc.sync.dma_start(stf4[32*q:32*q+32, cs:cs+TC],
                                      stf4[0:32, cs:cs+TC])
                st4_soi = stf4[:].rearrange("p (s i) -> p s i", i=O)
                psd2 = ps_C.tile([36, TC], f32, tag="psC", name="psd2")
                for half in range(2):
                    diff4 = p_df.tile([128, TC], f16, tag="df", name="diff4")
                    dsq4 = p_sq.tile([128, TC], f16, tag="sq", name="dsq4")
                    for q in range(4):
                        j = 4 * half + q
                        r0 = 32 * q
                        nc.vector.tensor_tensor(
                            diff4[r0:r0+32, :].rearrange("p (s i) -> p s i", i=O),
                            st4_soi[r0:r0+32, s0:s0+ns, :],
                            st4_soi[r0:r0+32, s0:s0+ns, j:j+1].broadcast_to((32, ns, O)),
                            op=OP.subtract)
                        nc.vector.tensor_tensor(dsq4[r0:r0+32, :], diff4[r0:r0+32, :],
                                                diff4[r0:r0+32, :], op=OP.mult)
                    mm(psd2[32*half:32*half+4, :], W["onesb4"][:], dsq4[:],
                       start=True, stop=True, skip_group_check=True)
                for half in range(2):
                    nc.scalar.activation(
                        aux[32*half:32*half+4, cs:cs+TC],
                        psd2[32*half:32*half+4, :], AF.Sqrt,
                        bias=W["eps36"][32*half:32*half+4, :])

              # ============ MAIN PASS (sigmoid table resident)
              for cc in range(n_chunks):
                cs = cc * TC
                s0 = cs // O
                ns = TC // O
                st_soi = stf4[0:32, :].rearrange("p (s i) -> p s i", i=O)

                # self-dynamics hidden
                ps_h = ps_B.tile([96, TC], f32, tag="psB", name="ps_h")
                mm(ps_h[:], W["wself0s"][:], stf4[0:32, cs:cs+TC], start=True, stop=True)
                hself = p_hs.tile([96, TC], f16, tag="hs", name="hself")
                evac_relu(0, hself[:], ps_h[:], W["sb0s"][:])

                # fill s_i rows (32:64) of both rhs buffers for this chunk (DMA)
                for b in range(2):
                    nc.sync.dma_start(rhs65[b][32:64, :], stf4[0:32, cs:cs+TC])

                dyn_acc = ps_D.tile([96, TC], f32, tag="psD", name="dyn_acc")

                for j in range(O):
                    rhs = rhs65[j % 2]
                    # partner block rows 0:32: s_j broadcast; dist row 64 via DMA
                    nc.gpsimd.tensor_copy(
                        rhs[0:32, :].rearrange("p (s i) -> p s i", i=O),
                        st_soi[:, s0:s0+ns, j:j+1].broadcast_to((32, ns, O)))
                    jr = j if j < 4 else 28 + j
                    nc.sync.dma_start(rhs[64:65, :], aux[jr:jr+1, cs:cs+TC])

                    a1s, a2s = [], []
                    a1_eng, a2_eng = (0, 1, 1), (1, 0, 0)
                    for c in range(3):
                        psA = ps_A.tile([128, TC], f32, tag="psA", name="psA")
                        mm(psA[:], W[f"w1p_{c}"][:], rhs[:], start=True, stop=True)
                        a1 = p_a1.tile([128, TC], f16, tag="a1", name="a1")
                        evac_relu(a1_eng[c], a1[:], psA[:], W[f"b1_{c}"][:])
                        a1s.append(a1)
                    for c in range(3):
                        psB = ps_B.tile([96, TC], f32, tag="psB", name="psB")
                        mm(psB[:], W[f"w2p_{c}"][:], a1s[c][:], start=True, stop=True)
                        a2 = p_a2.tile([96, TC], f16, tag="a2", name="a2")
                        evac_relu(a2_eng[c], a2[:], psB[:], W[f"b2_{c}"][:])
                        a2s.append(a2)
                    psC3 = ps_C.tile([96, TC], f32, tag="psC", name="psC3")
                    for c in range(3):
                        att0 = 32 * ((c + 1) % 3)
                        mm(psC3[32*c:32*c+32, :], W[f"w3a_{c}"][att0:att0+32, :],
                           a2s[c][att0:att0+32, :], start=True, stop=True,
                           skip_group_check=True)
                    alr3 = p_al.tile([96, TC], f16, tag="al", name="alr3")
                    nc.scalar.activation(alr3[:], psC3[:], AF.Sigmoid,
                                         bias=W["ab2r3"][:])
                    am3 = p_am.tile([96, TC], f16, tag="am", name="am3")
                    for c in range(3):
                        rel0 = 32 * c
                        eng = nc.gpsimd if c == 1 else nc.vector
                        eng.tensor_tensor(
                            am3[rel0:rel0+32, :], a2s[c][rel0:rel0+32, :],
                            alr3[rel0:rel0+32, :], op=OP.mult)
                    mm(dyn_acc[:], W["w3r3"][:], am3[:],
                       start=(j == 0), stop=False, skip_group_check=True)
                    mm(dyn_acc[:], W["w3rb"][:], alr3[:],
                       start=False, stop=False, skip_group_check=True)
                    # cancel the diagonal (i == j) columns exactly
                    am3_soi = am3[:].rearrange("p (s i) -> p s i", i=O)
                    alr3_soi = alr3[:].rearrange("p (s i) -> p s i", i=O)
                    dyn_soi = dyn_acc[:].rearrange("p (s i) -> p s i", i=O)
                    mm(dyn_soi[:, :, j], W["w3r3n"][:], am3_soi[:, :, j],
                       start=False, stop=False, skip_group_check=True)
                    mm(dyn_soi[:, :, j], W["w3rbn"][:], alr3_soi[:, :, j],
                       start=False, stop=False, skip_group_check=True)

                # self-dynamics into the same accumulator, then evacuate
                mm(dyn_acc[:], W["wself1bd"][:], hself[:],
                   start=False, stop=True, skip_group_check=True)
                dyn = p_ep.tile([96, TC], f16, tag="ep", name="dyn")
                evac_add(1, dyn[:], dyn_acc[:], W["dynb"][:])

                # ---- affector + out + agg chains
                cur = dyn
                for k in range(3):
                    psE = ps_B.tile([96, TC], f32, tag="psB", name="psE")
                    mm(psE[:], W[f"waff{k}bd"][:], cur[:], start=True, stop=True)
                    nxt = p_ep.tile([96, TC], f16, tag="ep", name="nxt")
                    if k < 2:
                        evac_relu(k % 2, nxt[:], psE[:], W[f"fb{k}s"][:])
                    else:
                        evac_add(0, nxt[:], psE[:], W[f"fb{k}s"][:])
                    cur = nxt
                psO = ps_A.tile([96, TC], f32, tag="psA", name="psO")
                mm(psO[:], W["wow0abd"][:], cur[:], start=True, stop=False)
                mm(psO[:], W["wow0ss"][:], stf4[0:32, cs:cs+TC], start=False, stop=True)
                o0 = p_ep.tile([96, TC], f16, tag="ep", name="o0")
                evac_relu(1, o0[:], psO[:], W["ob0s"][:])
                psO1 = ps_B.tile([96, TC], f32, tag="psB", name="psO1")
                mm(psO1[:], W["wow1bd"][:], o0[:], start=True, stop=True)
                ccat = p_ep.tile([96, TC], f16, tag="ep", name="ccat")
                nc.vector.tensor_copy(ccat[:], psO1[:])
                psG = ps_A.tile([32, TC], f32, tag="psA", name="psG")
                mm(psG[:], W["wagg1"][:], ccat[:], start=True, stop=True)
                h = p_ep.tile([32, TC], f16, tag="ep", name="h")
                evac_relu(0, h[:], psG[:], W["bagg1p"][:])
                psG2 = ps_B.tile([32, TC], f32, tag="psB", name="psG2")
                mm(psG2[:], W["wagg2"][:], h[:], start=True, stop=True)
                ot = p_out.tile([32, TC], f32, tag="ot", name="ot")
                evac_add(1, ot[:], psG2[:], W["bagg2"][:])
                nc.sync.dma_start(outT[:, cs:cs+TC], ot[:])

    nc.compile()
    return nc


# ---------------------------------------------------------------- host runner
_CACHE = {}


def _make_runner(nc, n_cores=N_CORES):
    import jax
    import numpy as _np
    import concourse.mybir as mybir
    from concourse import bass2jax
    from jax.sharding import Mesh, PartitionSpec
    from jax.experimental.shard_map import shard_map

    bass2jax.install_neuronx_cc_hook()
    partition_name = nc.partition_id_tensor.name if nc.partition_id_tensor else None
    in_names, out_names, out_avals, zero_shapes = [], [], [], []
    for alloc in nc.m.functions[0].allocations:
        if not isinstance(alloc, mybir.MemoryLocationSet):
            continue
        name = alloc.memorylocations[0].name
        if alloc.kind == "ExternalInput":
            if name != partition_name:
                in_names.append(name)
        elif alloc.kind == "ExternalOutput":
            out_names.append(name)
            shape = tuple(alloc.tensor_shape)
            dtype = mybir.dt.np(alloc.dtype)
            out_avals.append(jax.core.ShapedArray(shape, dtype))
            zero_shapes.append((shape, dtype))
    n_params = len(in_names)
    n_outs = len(out_avals)
    all_in_names = in_names + out_names + ([partition_name] if partition_name else [])
    donate = tuple(range(n_params, n_params + n_outs))

    def _body(*args):
        operands = list(args)
        if partition_name is not None:
            operands.append(bass2jax.partition_id_tensor())
        outs = bass2jax._bass_exec_p.bind(
            *operands, out_avals=tuple(out_avals), in_names=tuple(all_in_names),
            out_names=tuple(out_names), lowering_input_output_aliases=(),
            sim_require_finite=False, sim_require_nnan=False, nc=nc)
        return tuple(outs)

    devices = jax.devices()[:n_cores]
    mesh = Mesh(_np.asarray(devices), ("core",))
    sharded = jax.jit(
        shard_map(_body, mesh=mesh,
                  in_specs=(PartitionSpec("core"),) * (n_params + n_outs),
                  out_specs=(PartitionSpec("core"),) * n_outs,
                  check_rep=False),
        donate_argnums=donate, keep_unused=True)

    def run(in_maps):
        per_core = [[_np.asarray(m[name]) for name in in_names] for m in in_maps]
        concat_in = [_np.concatenate([per_core[c][i] for c in range(n_cores)], axis=0)
                     for i in range(n_params)]
        concat_zeros = [_np.zeros((n_cores * s[0], *s[1:]), d) for s, d in zero_shapes]
        out_arrs = sharded(*concat_in, *concat_zeros)
        jax.block_until_ready(out_arrs)
        return [
            {name: _np.asarray(out_arrs[i]).reshape(n_cores, *out_avals[i].shape)[c]
             for i, name in enumerate(out_names)}
            for c in range(n_cores)
        ]
    return run


def make_in_maps(inputs: dict, n_loc: int, n_cores: int = N_CORES):
    w = pack_weights(inputs)
    for k in _MMW:
        w[k] = w[k].astype(np.float16)
    z = np.asarray(inputs["z"], np.float32)
    in_maps = []
    for c in range(n_cores):
        zc = z[c*n_loc:(c+1)*n_loc].reshape(n_loc * O, D_IN)
        m = dict(w)
        m["zT"] = np.ascontiguousarray(zc.T).astype(np.float16)
        in_maps.append(m)
    return in_maps


def kernel(**inputs) -> np.ndarray:
    n = inputs["z"].shape[0]
    n_loc = n // N_CORES
    key = ("k", n_loc)
    if key not in _CACHE:
        nc = build_nc(n_loc)
        _CACHE[key] = (nc, _make_runner(nc))
    nc, runner = _CACHE[key]
    res = runner(make_in_maps(inputs, n_loc))
    out = np.concatenate(
        [res[c]["outT"].T.reshape(n_loc, O, CL) for c in range(N_CORES)], axis=0)
    return out
